# revision 12
# baseline (speedup 1.0000x reference)
"""Trainium2 Bass kernel for truncated-exp kNN label regression.

Reference computation (B=4096 queries, K=64 neighbors, N=100000 exemplars,
L=256 label dim):
    w      = exp(-sq_dists) * (sq_dists <= tau^2)                        [B, K]
    numer  = sum_k w[b,k] * exemplar_labels[labels[b,k]] + gamma_n*avg   [B, L]
    denom  = sum_k w[b,k] * exemplar_sizes[labels[b,k]] + gamma_n        [B]
    out    = numer / denom                                               [B, L]

Strategy: data-parallel over B across 8 NeuronCores (512 queries each).
Host-side prep:
  - augmented table [N, 257]: row = [labels_row(256) | size] so one gathered
    row feeds both numerator and denominator,
  - per core, sq_dists/labels [512, 64] retiled to [128, 4*64] (partition =
    query % 128, free = qtile*64 + k),
  - avg extended to [1, 257] with trailing 1.0 (the denominator's gamma seed).
Device per core (all natural layouts, no transposes):
  - w[128, 256] = exp(-sqd)*(sqd<=tau2) elementwise,
  - 8 indirect DMAs per qtile gather 8 rows/partition each (row for
    (query p, neighbor k) lands on partition p) from the augmented table,
  - each gathered [128, 257] chunk is scaled in place by its weight column
    (per-partition tensor_scalar), then accumulated into PSUM with an
    identity-lhsT matmul (PE as per-partition accumulator),
  - psum_t[128, 257] initialized with gamma_n*[avg|1] via a 1-contract
    broadcast matmul,
  - epilogue: out = psum[:, :256] * (1/psum[:, 256]), DMA out.
"""

import numpy as np

from concourse import bacc, bass, mybir
import concourse.tile as tile
from concourse.bass_utils import run_bass_kernel_spmd
from concourse.masks import make_identity

B, K, N, L = 4096, 64, 100000, 256
NCORES = 8
BS = B // NCORES            # 512 queries per core
QT = BS // 128              # 4 query tiles of 128 queries
R = L + 1                   # 257: gathered row = labels(256) + size(1)
GM = 1                      # rows gathered per partition per indirect DMA
NG = K // GM                # 8 gather groups per query tile


def _build(tau2: float, gamma_n: float) -> bass.Bass:
    nc = bacc.Bacc()
    f32 = mybir.dt.float32
    i32 = mybir.dt.int32

    sqd = nc.dram_tensor("sqd", [128, QT * K], f32, kind="ExternalInput")
    lab = nc.dram_tensor("lab", [128, QT * K], i32, kind="ExternalInput")
    table = nc.dram_tensor("table", [N, R], f32, kind="ExternalInput")
    avg = nc.dram_tensor("avg", [1, R], f32, kind="ExternalInput")
    out_d = nc.dram_tensor("out", [BS, L], f32, kind="ExternalOutput")

    with tile.TileContext(nc) as tc:
        with (
            tc.tile_pool(name="const", bufs=1) as cpool,
            tc.tile_pool(name="io", bufs=1) as iopool,
            tc.tile_pool(name="gather", bufs=3) as gpool,
            tc.tile_pool(name="gw", bufs=6) as wpool,
            tc.tile_pool(name="outp", bufs=2) as opool,
            tc.tile_pool(name="psum", bufs=4, space="PSUM") as ppool,
        ):
            sqd_t = iopool.tile([128, QT * K], f32)
            nc.sync.dma_start(out=sqd_t[:], in_=sqd[:, :])
            lab_t = iopool.tile([128, QT * K], i32)
            nc.sync.dma_start(out=lab_t[:], in_=lab[:, :])
            avg_t = cpool.tile([1, R], f32)
            nc.sync.dma_start(out=avg_t[:1, :], in_=avg[:1, :])

            ident = cpool.tile([128, 128], f32)
            make_identity(nc, ident[:, :])
            ones_t = cpool.tile([1, 128], f32)
            nc.vector.memset(ones_t[:1, :], 1.0)
            # ga = gamma_n * [avg | 1.0]
            ga_t = cpool.tile([1, R], f32)
            nc.scalar.activation(
                out=ga_t[:1, :], in_=avg_t[:1, :],
                func=mybir.ActivationFunctionType.Copy, scale=gamma_n,
            )

            # w = exp(-sqd) * (sqd <= tau2)
            e_t = iopool.tile([128, QT * K], f32)
            nc.scalar.activation(
                out=e_t[:], in_=sqd_t[:],
                func=mybir.ActivationFunctionType.Exp, scale=-1.0,
            )
            m_t = iopool.tile([128, QT * K], f32)
            nc.vector.tensor_scalar(
                out=m_t[:], in0=sqd_t[:], scalar1=tau2, scalar2=None,
                op0=mybir.AluOpType.is_le,
            )
            w_t = iopool.tile([128, QT * K], f32)
            nc.vector.tensor_mul(out=w_t[:], in0=e_t[:], in1=m_t[:])

            for t in range(QT):
                ps = ppool.tile([128, R], f32)
                # psum init: every partition gets gamma_n * [avg | 1]
                nc.tensor.matmul(
                    out=ps[:], lhsT=ones_t[:1, :], rhs=ga_t[:1, :],
                    start=True, stop=False, skip_group_check=True,
                )

                for g in range(NG):
                    gt = gpool.tile([128, GM * R], f32)
                    nc.gpsimd.indirect_dma_start(
                        out=gt[:],
                        out_offset=None,
                        in_=table[:, :],
                        in_offset=bass.IndirectOffsetOnAxis(
                            ap=lab_t[:, t * K + g * GM:t * K + (g + 1) * GM],
                            axis=0,
                        ),
                    )
                    for i in range(GM):
                        k = g * GM + i
                        # scale row (query p, neighbor k) by w[p, k]
                        gw = wpool.tile([128, R], f32)
                        nc.vector.tensor_scalar(
                            out=gw[:],
                            in0=gt[:, i * R:(i + 1) * R],
                            scalar1=w_t[:, t * K + k:t * K + k + 1],
                            scalar2=None,
                            op0=mybir.AluOpType.mult,
                        )
                        # psum += I @ gw  (PE as per-partition accumulator)
                        nc.tensor.matmul(
                            out=ps[:],
                            lhsT=ident[:, :],
                            rhs=gw[:],
                            start=False, stop=(k == K - 1),
                            skip_group_check=True,
                        )

                rec = opool.tile([128, 1], f32)
                nc.vector.reciprocal(out=rec[:], in_=ps[:, L:L + 1])
                ot = opool.tile([128, L], f32)
                nc.vector.tensor_scalar(
                    out=ot[:], in0=ps[:, :L], scalar1=rec[:, :1],
                    scalar2=None, op0=mybir.AluOpType.mult,
                )
                nc.sync.dma_start(
                    out=out_d[t * 128:(t + 1) * 128, :], in_=ot[:],
                )
    nc.finalize()
    return nc


def _host_prep(inputs):
    sq = np.asarray(inputs["sq_dists"], dtype=np.float32)
    labels = np.asarray(inputs["labels"]).astype(np.int32)
    tab = np.asarray(inputs["exemplar_labels"], dtype=np.float32)
    sizes = np.asarray(inputs["exemplar_sizes"], dtype=np.float32)
    avg = np.asarray(inputs["average_label"], dtype=np.float32)
    tau2 = float(np.asarray(inputs["tau_squared"]))
    gamma_n = float(np.asarray(inputs["gamma_n"]))

    table = np.ascontiguousarray(
        np.concatenate([tab, sizes[:, None]], axis=1)
    )  # [N, 257]
    avg_ext = np.concatenate([avg, np.ones(1, np.float32)])[None, :].astype(
        np.float32
    )  # [1, 257]

    def retile(x):  # [512, 64] -> [128, 4*64], partition = query % 128
        return np.ascontiguousarray(
            x.reshape(QT, 128, K).transpose(1, 0, 2).reshape(128, QT * K)
        )

    in_maps = []
    for c in range(NCORES):
        s = sq[c * BS:(c + 1) * BS]
        l = labels[c * BS:(c + 1) * BS]
        in_maps.append({
            "sqd": retile(s),
            "lab": retile(l),
            "table": table,
            "avg": avg_ext,
        })
    return in_maps, tau2, gamma_n


def _run(inputs, trace: bool = False):
    in_maps, tau2, gamma_n = _host_prep(inputs)
    nc = _build(tau2, gamma_n)
    res = run_bass_kernel_spmd(nc, in_maps, list(range(NCORES)), trace=trace)
    out = np.concatenate(
        [res.results[c]["out"] for c in range(NCORES)], axis=0
    ).astype(np.float32)
    return out, res


def kernel(**inputs) -> np.ndarray:
    out, _ = _run(inputs, trace=False)
    return out
